# revision 58
# baseline (speedup 1.0000x reference)
"""MoE kernel for TRN2, 8 NeuronCores, data-parallel over the batch dim.

Reference computation (B=8192, D=1024, H=1024, E=16):
    weights = softmax(x @ Wg + bg, axis=1)            # [B, E]
    h       = relu(einsum('bd,edh->beh', x, W1) + b1) # [B, E, H]
    eo      = einsum('beh,eh->be', h, W2) + b2        # [B, E]
    out     = sum(eo * weights, axis=1, keepdims=True)# [B, 1]

Strategy (618us stub -> 481us bf16 -> ~440us with partial fp8):
  - Shard B over 8 cores (1024 rows/core); weights replicated.
  - bf16 on the PE everywhere except a PARTIAL-fp8 stage 1 (below):
    bf16 runs 1 col/cycle with fast weight load, stage-1 matmuls issue
    at the ideal ~216ns/N=512 spacing at the warm 2.4GHz clock.
  - Partial fp8: for the 12 experts with the smallest mean squared gate
    weight (expert 0 always excluded), the last d-tile PAIR (d6,d7) of
    every stage-1 accumulation runs as ONE fp8e4 DoubleRow matmul
    (2 MACs/cell/cycle, ~1.8x): 3/16 of the contraction in e4m3.
    x/16 and W1*16 keep both operands in e4m3's normal range (W1 is
    half subnormal raw) while the product is exact. Measured rel err
    1.58e-2 vs the 2e-2 tolerance (bf16 alone: 3.9e-3); saves ~36us.
  - Stage 1 per t=(ht,e): two psum tiles [h=128, b=512] (split per
    batch half so the recycled buffer's WAR only waits the matching
    ReLU chunk); accumulated over 8 d-tiles (6 bf16 + 1 DoubleRow for
    fp8 tiles); W1 streamed on the sync ring 8 tiles deep.
  - ReLU+b1 on ScalarE -> hr bf16.
  - Stage 2 uses PE column tiling: experts are assigned to the 4 32-col
    groups (e -> group e%4, row e//4), so consecutive t's stage-2
    matmuls run CONCURRENTLY in disjoint col groups (~4x faster than a
    serialized block-diagonal form). Emitted in bursts of 64 t's; the
    eo accumulator is SPLIT into one psum tile per batch half so the
    combine's V-mul reads of half A don't block (tile-granular WAR)
    the stage-2 writes of half B.
  - Gating: stationary Wg produces logits directly as [16e, B] (expert
    e at partition 32*(e%4)+e//4); U = exp(logits + bg) UNNORMALIZED.
    den = sum_e U and num2 = sum_e U*b2 via tiny partition-sum matmuls.
    Gating matmuls are interleaved with stage-1 t=0 in xt-chunk arrival
    order; xt is split across both hardware DGE rings (scalar+sync;
    gpsimd's software DGE has ~us issue latency — avoid).
  - 40 x 128-col dependency-free warm-up matmuls bridge the ~4.5us DMA
    pipe latency so the PE HAM clock-gate is at full rate (2.4GHz) and
    the activity window never lapses before real work arrives.
  - Combine: V = U * eo (DVE, quarter-chased behind the final flush,
    first half overlapped with the second half's stage-2 backlog) ->
    num1 via 8 partition-sum matmuls -> y = (num1 + num2) / den -> one
    [128,8] DMA. (num1 must NOT accumulate onto num2's psum: start=True
    clears has_written bank-wide, so those bits are long gone.)
"""

import ml_dtypes
import numpy as np

import concourse.bacc as bacc
import concourse.bass as bass
import concourse.mybir as mybir
from concourse import tile
from concourse.bass_utils import run_bass_kernel_spmd

B, D, H, E = 8192, 1024, 1024, 16
N_CORES = 8
BS = B // N_CORES  # 1024 batch rows per core
BH = 512           # half-batch moving-operand width (one psum bank)
DT = D // 128      # 8 d-tiles
HT = H // 128      # 8 h-tiles
T = E * HT         # 128 tiles; t = ht*16 + e  (e minor)
NCH = BS // 128    # 8 b-chunks of 128

F32 = mybir.dt.float32
BF16 = mybir.dt.bfloat16
F8 = mybir.dt.float8e4
AF = mybir.ActivationFunctionType
BF_NP = ml_dtypes.bfloat16
F8_NP = ml_dtypes.float8_e4m3
# fp8 stage-1: tiles of the 12 selected experts compute their last
# d-tile PAIR (d6,d7) as one fp8e4 DoubleRow matmul (2 MACs/cell/cycle)
# -> 3/16 of the contraction in e4m3, ~1.55e-2 rel err (tolerance 2e-2),
# ~36us of PE time saved. Scales x/16 and W1*16 so both operands sit in
# e4m3's normal range (W1 ~ +-1/32 is half subnormal raw); the product
# scale cancels exactly so it accumulates directly with the bf16 psum.
XS = 16.0


def build_bass(es_f8):
    """es_f8: the experts (12 of 16, never expert 0) whose tiles compute
    the (d6,d7) pair in fp8 DoubleRow."""
    n_f8 = 8 * len(es_f8)
    n_bf = T - n_f8
    nc = bacc.Bacc("TRN2", target_bir_lowering=False, debug=False)
    xt_d = nc.dram_tensor("xt", [128, DT * BS], BF16, kind="ExternalInput")
    xt8_d = nc.dram_tensor("xt8", [128, 2, BS], F8, kind="ExternalInput")
    w1e_d = nc.dram_tensor("w1e", [n_bf, 128, DT * 128], BF16,
                           kind="ExternalInput")
    w1o_d = nc.dram_tensor("w1o", [n_f8, 128, 6 * 128], BF16,
                           kind="ExternalInput")
    w8_d = nc.dram_tensor("w8", [n_f8, 128, 2, 128], F8,
                          kind="ExternalInput")
    b1t_d = nc.dram_tensor("b1t", [128, T], F32, kind="ExternalInput")
    w2c_d = nc.dram_tensor("w2c", [128, T * 32], BF16, kind="ExternalInput")
    wgp_d = nc.dram_tensor("wgp4", [128, DT * 128], BF16, kind="ExternalInput")
    bg4_d = nc.dram_tensor("bg4", [128, 1], F32, kind="ExternalInput")
    ob2_d = nc.dram_tensor("ob2", [128, 2], BF16, kind="ExternalInput")
    y_d = nc.dram_tensor("y", [128, NCH], F32, kind="ExternalOutput")

    with tile.TileContext(nc) as tc:
        with (
            tc.tile_pool(name="const", bufs=1) as cpool,
            tc.tile_pool(name="w1", bufs=4) as w1pool,
            tc.tile_pool(name="w1o", bufs=6) as w1opool,
            tc.tile_pool(name="w8", bufs=6) as w8pool,
            tc.tile_pool(name="hrelu", bufs=66) as hpool,
            tc.tile_pool(name="sm", bufs=2) as smpool,
            # ps1A gets a 3rd buffer (the last free PSUM bank) so a new
            # t's start=True matmul never waits on ReLU two t's back
            tc.tile_pool(name="ps_hA", bufs=3, space=bass.MemorySpace.PSUM) as psA,
            tc.tile_pool(name="ps_hB", bufs=2, space=bass.MemorySpace.PSUM) as psB,
            tc.tile_pool(name="ps_eo", bufs=1, space=bass.MemorySpace.PSUM) as pseo,
            tc.tile_pool(name="ps_s", bufs=1, space=bass.MemorySpace.PSUM) as pss,
        ):
            # ---- resident tensors; xt is split across BOTH HWDGE rings.
            # scalar=ACT ring carries xt d0..3 (nothing ahead of them);
            # sync=SP ring carries wgp, w1[0], then xt d4..7 ahead of the
            # W1 stream. First gating MM (needs wgp + xt d0) can start
            # ~0.7us after user code begins. Transfers are ~0.67us each at
            # ~190GB/s per ring. ----
            wgp_sb = cpool.tile([128, DT * 128], BF16, tag="wgp")
            nc.sync.dma_start(wgp_sb[:], wgp_d[:])
            w1_first = w1pool.tile([128, DT * 128], BF16, tag="w1t")
            nc.sync.dma_start(w1_first[:], w1e_d[0, :, :])
            # xt on the two hardware DGE rings (gpsimd's software DGE has
            # ~us issue latency and delays the warm-up memset behind it)
            xt_sb = cpool.tile([128, DT * BS], BF16, tag="xt")
            for dd in range(DT):
                eng = nc.scalar if dd < 4 else nc.sync
                eng.dma_start(
                    xt_sb[:, dd * BS:(dd + 1) * BS], xt_d[:, dd * BS:(dd + 1) * BS]
                )
            xt8_sb = cpool.tile([128, 2, BS], F8, tag="xt8")
            nc.scalar.dma_start(xt8_sb[:], xt8_d[:])
            b1t_sb = cpool.tile([128, T], F32, tag="b1t")
            nc.scalar.dma_start(b1t_sb[:], b1t_d[:])
            # w2c split in halves: the scheduler eagerly hoists the first
            # stage-2 matmuls right after t=0's ReLU (~25us) — the first
            # half (t<64 weights) must be resident by then; the rest can
            # trail everything else
            w2c_sb = cpool.tile([128, T * 32], BF16, tag="w2c")
            nc.scalar.dma_start(w2c_sb[:, :T * 16], w2c_d[:, :T * 16])
            bg4_sb = cpool.tile([128, 1], F32, tag="bg4")
            nc.scalar.dma_start(bg4_sb[:], bg4_d[:])
            ob2_sb = cpool.tile([128, 2], BF16, tag="ob2")
            nc.scalar.dma_start(ob2_sb[:], ob2_d[:])
            nc.scalar.dma_start(w2c_sb[:, T * 16:], w2c_d[:, T * 16:])

            # gating is interleaved with stage-1 t=0 inside the main loop.
            # gps lives in the eo pool (bufs=1, so it aliases eo_ps): the
            # Exp read finishes by ~21us and eo's first burst (t=8, all
            # start=True) fully reinitializes the banks — and this keeps
            # ps1's two psh buffers free of WAR stalls against Exp.
            # eo accumulator is SPLIT per batch half (two psum tiles) so
            # the tile tracker sees the V-mul reads of half A and the
            # stage-2 writes of half B as independent (no WAR stall).
            # gps aliases it (pseo bufs=1): Exp reads finish early and the
            # first stage-2 burst's start=True reinitializes the banks.
            gpsA = pseo.tile([128, BH], F32, tag="eoA")
            gpsB = pseo.tile([128, BH], F32, tag="eoB")
            gps2 = (gpsA, gpsB)
            # consume xt chunks roughly in DMA-arrival order
            # (scalar: d0..3 back-to-back; sync: wgp, w1[0], d4..7)
            gate_order = [0, 1, 4, 2, 5, 3, 6, 7]
            u4 = cpool.tile([128, BS], BF16, tag="u4")
            u4f = cpool.tile([128, BS], F32, tag="u4f")

            # HAM warm-up: dependency-free matmuls cover the DMA pipe
            # latency (~3.5us issue-to-completion for the first wgp/xt
            # chunks, landing ~10.7us) so the PE clock is warm (8/8) and
            # busy until real work can start. 34 x 128-col at the cold
            # rate (~107ns) spans ~3.6us. Results are overwritten by the
            # gating group's start=True.
            scratch = cpool.tile([128, 128], BF16, tag="scratch")
            nc.gpsimd.memset(scratch[:], 0.0)
            for _ in range(40):
                nc.tensor.matmul(
                    gpsA[:, 0:128],
                    scratch[:],
                    scratch[:],
                    start=True, stop=True, skip_group_check=True,
                )

            # ---- main loop over t = ht*16 + e ----
            eoA = pseo.tile([128, BH], F32, tag="eoA")
            eoB = pseo.tile([128, BH], F32, tag="eoB")
            eo2 = (eoA, eoB)
            pending = []  # [(t, hr), ...] up to BURST
            BURST = 64

            def emit_stage2_burst():
                # consecutive t's alternate col groups, so MMs overlap
                for bh in range(2):
                    for (tt, hh) in pending:
                        g = (tt % E) % 4
                        nc.tensor.matmul(
                            eo2[bh][32 * g:32 * g + 32, :],
                            w2c_sb[:, tt * 32:(tt + 1) * 32],
                            hh[:, bh * BH:(bh + 1) * BH],
                            start=(tt < 4), stop=(tt >= T - 4),
                            skip_group_check=True,
                            tile_position=(0, 32 * g),
                        )
                pending.clear()

            # den/nb2/num packed into ONE psum tile (one bank): psum pool
            # allocation is bank-granular per tag, and all 8 banks are
            # spoken for. Values survive the bank-wide has_written churn
            # because each column is written exactly once with start=True.
            sums_ps = pss.tile([128, 3 * NCH], F32, tag="sps")
            rden = cpool.tile([128, NCH], F32, tag="rden")
            num2 = cpool.tile([128, NCH], F32, tag="num2")

            ibf = if8 = 0  # per-stream tile counters
            for t in range(T):
                fp8_t = (t % E) in es_f8
                if t == 0:
                    w1t = w1_first
                    ibf += 1
                elif fp8_t:
                    w1t = w1opool.tile([128, 6 * 128], BF16, tag="w1o")
                    nc.sync.dma_start(w1t[:], w1o_d[if8, :, :])
                    w8t = w8pool.tile([128, 2, 128], F8, tag="w8t")
                    nc.sync.dma_start(w8t[:], w8_d[if8, :, :, :])
                    if8 += 1
                else:
                    w1t = w1pool.tile([128, DT * 128], BF16, tag="w1t")
                    nc.sync.dma_start(w1t[:], w1e_d[ibf, :, :])
                    ibf += 1
                # ps1 split per batch half so a new t's start=True matmul
                # (WAR on the recycled psum buffer) only waits for the
                # matching ReLU chunk, not both
                ps1A = psA.tile([128, BH], F32, tag="ps1A")
                ps1B = psB.tile([128, BH], F32, tag="ps1B")
                ps12 = (ps1A, ps1B)
                # t=0: follow the two xt DMA streams, interleaving the
                # gating matmuls so the PE has work as tiles arrive
                dds = gate_order if t == 0 else range(6 if fp8_t else DT)
                for i, dd in enumerate(dds):
                    if t == 0:
                        for bh in range(2):
                            nc.tensor.matmul(
                                gps2[bh][:, :],
                                wgp_sb[:, dd * 128:(dd + 1) * 128],
                                xt_sb[:, dd * BS + bh * BH: dd * BS + (bh + 1) * BH],
                                start=(i == 0), stop=(i == DT - 1),
                                skip_group_check=True,
                            )
                    lhs = w1t[:, dd * 128:(dd + 1) * 128]
                    for bh in range(2):
                        nc.tensor.matmul(
                            ps12[bh][:, :],
                            lhs,
                            xt_sb[:, dd * BS + bh * BH: dd * BS + (bh + 1) * BH],
                            start=(i == 0),
                            stop=(not fp8_t and i == DT - 1),
                            skip_group_check=True,
                        )
                if fp8_t:
                    # d6+d7 fused: fp8e4 DoubleRow, 256-deep contraction
                    for bh in range(2):
                        nc.tensor.matmul(
                            ps12[bh][:, :],
                            w8t[:, 0:2, :],
                            xt8_sb[:, 0:2, bh * BH:(bh + 1) * BH],
                            start=False, stop=True,
                            perf_mode=mybir.MatmulPerfMode.DoubleRow,
                            skip_group_check=True,
                        )
                if t == 0:
                    for bh in range(2):
                        nc.scalar.activation(
                            u4[:, bh * BH:(bh + 1) * BH], gps2[bh][:, :],
                            AF.Exp, bias=bg4_sb[:],
                        )
                    nc.vector.tensor_copy(u4f[:], u4[:])
                if len(pending) == BURST:
                    emit_stage2_burst()
                if t == 1:
                    # den/num2 partition-sums; u4 is ready by now, PE is warm
                    for j in range(NCH):
                        nc.tensor.matmul(
                            sums_ps[:, j:j + 1],
                            u4[:, j * 128:(j + 1) * 128],
                            ob2_sb[:, 0:1],
                            start=True, stop=True, skip_group_check=True,
                        )
                        nc.tensor.matmul(
                            sums_ps[:, NCH + j:NCH + j + 1],
                            u4[:, j * 128:(j + 1) * 128],
                            ob2_sb[:, 1:2],
                            start=True, stop=True, skip_group_check=True,
                        )
                if t == 2:
                    nc.vector.reciprocal(rden[:], sums_ps[:, 0:NCH])
                    nc.vector.tensor_copy(num2[:], sums_ps[:, NCH:2 * NCH])
                hr = hpool.tile([128, BS], BF16, tag="hr")
                # final tile: quarter-width ReLU chunks so the last stage-2
                # matmuls can chase them with minimal PE wait
                nq = 4 if t == T - 1 else 2
                w = BS // nq
                for q in range(nq):
                    src = ps12[q * w // BH]
                    off = (q * w) % BH
                    nc.scalar.activation(
                        hr[:, q * w:(q + 1) * w],
                        src[:, off:off + w],
                        AF.Relu,
                        bias=b1t_sb[:, t:t + 1],
                    )
                if t < T - 1:
                    pending.append((t, hr))
                else:
                    hr_last = hr

            # ---- final flush + combine, interleaved so the V-mul and
            # num1 partition-sums for the first batch half run while the
            # PE is still flushing the second half's stage-2 backlog.
            # (num1 gets a fresh psum region: a start=False accumulate
            # onto nb2_ps would OVERWRITE num2 — start=True clears the
            # has_written bits bank-wide, so only the last t=1 column
            # still has its bit set by the end.) ----
            v4 = cpool.tile([128, BS], BF16, tag="v4")

            g_last = ((T - 1) % E) % 4

            def chase(q):
                bh, qq = divmod(q, 2)
                nc.tensor.matmul(
                    eo2[bh][32 * g_last:32 * g_last + 32, qq * 256:(qq + 1) * 256],
                    w2c_sb[:, (T - 1) * 32:T * 32],
                    hr_last[:, q * 256:(q + 1) * 256],
                    start=False, stop=True,
                    skip_group_check=True,
                    tile_position=(0, 32 * g_last),
                )

            def vmul(q):
                bh, qq = divmod(q, 2)
                loc = slice(qq * 256, (qq + 1) * 256)
                cols = slice(q * 256, (q + 1) * 256)
                nc.vector.tensor_mul(v4[:, cols], eo2[bh][:, loc], u4f[:, cols])

            def num1(j):
                nc.tensor.matmul(
                    sums_ps[:, 2 * NCH + j:2 * NCH + j + 1],
                    v4[:, j * 128:(j + 1) * 128],
                    ob2_sb[:, 0:1],
                    start=True, stop=True, skip_group_check=True,
                )

            def flush_half(bh, interleave=()):
                # interleave: {flush position: [num1 col, ...]} — tuck the
                # num1 partition-sums between backlog matmuls late enough
                # that their V inputs are ready (no PE stall)
                for k, (tt, hh) in enumerate(pending):
                    g = (tt % E) % 4
                    nc.tensor.matmul(
                        eo2[bh][32 * g:32 * g + 32, :],
                        w2c_sb[:, tt * 32:(tt + 1) * 32],
                        hh[:, bh * BH:(bh + 1) * BH],
                        start=False, stop=(tt >= T - 4),
                        skip_group_check=True,
                        tile_position=(0, 32 * g),
                    )
                    for j in dict(interleave).get(k, ()):
                        num1(j)

            flush_half(0)
            chase(0)
            chase(1)
            vmul(0)
            vmul(1)
            flush_half(1, interleave={30: (0, 1), 40: (2, 3)})
            pending.clear()
            chase(2)
            chase(3)
            vmul(2)
            vmul(3)
            for j in range(4, 8):
                num1(j)
            ysb = smpool.tile([128, NCH], F32, tag="ysb")
            nc.vector.tensor_add(ysb[:], sums_ps[:, 2 * NCH:3 * NCH], num2[:])
            nc.vector.tensor_mul(ysb[:], ysb[:], rden[:])
            nc.sync.dma_start(y_d[:], ysb[:])
    nc.compile()
    return nc


def pick_f8_experts(x, Wg, bg):
    """The 4 experts with largest mean squared gate weight stay bf16
    (their output error matters most); expert 0 is also kept bf16 so the
    t=0 startup path stays uniform. Returns the 12 fp8 experts."""
    logits = np.asarray(x, np.float32) @ np.asarray(Wg, np.float32) + bg
    w = np.exp(logits - logits.max(1, keepdims=True))
    w /= w.sum(1, keepdims=True)
    w2m = (w * w).mean(0)
    keep = {0}
    for e in np.argsort(-w2m):
        if len(keep) >= 4:
            break
        keep.add(int(e))
    return sorted(set(range(E)) - keep)


def prep_inputs(x, W1, b1, W2, b2, Wg, bg):
    """Host-side data prep. Returns (shared, xts, xts8, es_f8)."""
    f = np.float32
    es_f8 = pick_f8_experts(x, Wg, bg)
    f8_set = set(es_f8)
    # W1 [E, D, H] -> [t=(ht,e), d_in, (d_t, h_in)]
    w1p = np.asarray(
        W1.reshape(E, DT, 128, HT, 128).transpose(3, 0, 2, 1, 4)
        .reshape(T, 128, DT * 128), dtype=f)
    is_f8 = np.array([(t % E) in f8_set for t in range(T)])
    w1e = np.ascontiguousarray(w1p[~is_f8]).astype(BF_NP)
    w1o = np.ascontiguousarray(w1p[is_f8][:, :, :6 * 128]).astype(BF_NP)
    # fp8 tiles' (d6,d7) pair, scaled into e4m3 normal range:
    # [n_f8, p, 2, 128]
    w8 = np.ascontiguousarray(
        (w1p[is_f8][:, :, 6 * 128:] * XS).reshape(-1, 128, 2, 128)
    ).astype(F8_NP)
    b1t = np.ascontiguousarray(
        b1.reshape(E, HT, 128).transpose(2, 1, 0).reshape(128, T).astype(f))
    # stage-2 stationaries: expert e -> col group g=e%4, row k=e//4
    w2c = np.zeros((128, T, 32), dtype=f)
    for t in range(T):
        ht, e = divmod(t, E)
        k, g = divmod(e, 4)
        w2c[:, t, k] = W2[e, ht * 128:(ht + 1) * 128]
    w2c = w2c.reshape(128, T * 32).astype(BF_NP)
    # gating stationary: col 32g+k = Wg[:, 4k+g], rest zero
    wgp4 = np.zeros((DT, 128, 128), dtype=f)
    bg4 = np.full((128, 1), -30.0, dtype=f)
    ob2 = np.zeros((128, 2), dtype=f)
    ob2[:, 0] = 1.0
    for e in range(E):
        k, g = divmod(e, 4)
        wgp4[:, :, 32 * g + k] = Wg[:, e].reshape(DT, 128)
        bg4[32 * g + k, 0] = bg[e]
        ob2[32 * g + k, 1] = b2[e]
    wgp4 = np.ascontiguousarray(
        wgp4.transpose(1, 0, 2).reshape(128, DT * 128)).astype(BF_NP)
    ob2 = ob2.astype(BF_NP)
    shared = {"w1e": w1e, "w1o": w1o, "w8": w8, "b1t": b1t, "w2c": w2c,
              "wgp4": wgp4, "bg4": bg4, "ob2": ob2}
    xT = np.ascontiguousarray(np.asarray(x, dtype=f).T)  # [D, B]
    xts, xts8 = [], []
    for c in range(N_CORES):
        xc = xT[:, c * BS:(c + 1) * BS]  # [D, BS]
        xc = np.ascontiguousarray(
            xc.reshape(DT, 128, BS).transpose(1, 0, 2).reshape(128, DT * BS))
        xts.append(xc.astype(BF_NP))
        xc8 = (xT[6 * 128:, c * BS:(c + 1) * BS] / XS)  # [256, BS]
        xc8 = np.ascontiguousarray(
            xc8.reshape(2, 128, BS).transpose(1, 0, 2))  # [128, 2, BS]
        xts8.append(xc8.astype(F8_NP))
    return shared, xts, xts8, es_f8


def run(inputs, trace=False):
    shared, xts, xts8, es_f8 = prep_inputs(**inputs)
    nc = build_bass(es_f8)
    in_maps = [dict(shared, xt=xts[c], xt8=xts8[c]) for c in range(N_CORES)]
    res = run_bass_kernel_spmd(
        nc, in_maps, core_ids=list(range(N_CORES)), trace=trace
    )
    # y dram is [128, NCH] with y[p, j] = out[j*128 + p]
    y = np.concatenate(
        [np.asarray(r["y"], dtype=np.float32).T.reshape(BS, 1)
         for r in res.results], axis=0)
    return y, res


def kernel(**inputs):
    y, _ = run(inputs, trace=False)
    return y


if __name__ == "__main__":
    rng = np.random.default_rng(0)
    ins = {
        "x": rng.standard_normal((B, D), dtype=np.float32),
        "W1": rng.standard_normal((E, D, H), dtype=np.float32) / 32,
        "b1": rng.standard_normal((E, H), dtype=np.float32) / 32,
        "W2": rng.standard_normal((E, H), dtype=np.float32) / 32,
        "b2": rng.standard_normal((E,), dtype=np.float32) / 32,
        "Wg": rng.standard_normal((D, E), dtype=np.float32) / 32,
        "bg": rng.standard_normal((E,), dtype=np.float32) / 32,
    }
    y = kernel(**ins)
    print("ok", y.shape, y.dtype)

